# revision 26
# baseline (speedup 1.0000x reference)
"""Trainium2 Bass kernel for nn_DenseGCM (scatter_memory).

Reference semantics (B=64, N=1024, F=64):
    of = (num_nodes + 1) > N            # wrap_overflow -- structurally dead:
                                        # num_nodes ~ randint(0, N) <= N-1
    nodes_in  = nodes with row num_nodes[b] <- x[b]
    nodes_out = nodes_in + posenc * (n <= num_nodes[b])
    agg       = adj @ nodes_in
    mx        = tanh(agg @ W)[b, num_nodes[b]]
    returns (mx, nodes_out, adj, weights, num_nodes + 1)

Only one row of the [B,N,N]x[B,N,F] einsum is observable through mx:
    mx[b] = tanh((adj[b, nn_b, :] @ nodes_in[b]) @ W)
so the device computes that row's aggregation plus the full masked
positional-encoding add over nodes.  adj / weights are exact passthroughs;
the tiny (agg+corr)@W -> tanh tail and the 64-row scatter are applied on
the host during unsharding.

Sharding: pure data parallel, 8 batches per core across 8 NeuronCores.
All data-dependent quantities (mask columns, adj rows) are passed as
per-core input tensors so one SPMD program serves all cores.

Device-side layout: nodes are host-pre-tiled so each DMA moves 2-4 KiB
contiguous runs per partition (line rate) while keeping partition = node
row within a 128-chunk (what the matmul contraction needs).  DRAM row
r = p*64 + b*8 + t of the [8192, 64] tensor holds nodes[b, t*128+p, :].

The per-batch aggregation uses the adjacency rows as the *stationary*
matmul operand ([128, 8]: ldweights cost scales with columns) and node
chunks as moving operands, spread over 4 concurrent TensorE column tiles
(128x32 array mode, tile_position).  The lhsT slice for (b, t) starts at
adjt column t*8+b, putting batch b's adjrow in lhsT column 0, so every
psum region's valid row is its row 0 (at partitions {0,32,64,96}).
"""

from contextlib import ExitStack

import numpy as np

import concourse.bacc as bacc
import concourse.bass as bass
import concourse.mybir as mybir
import concourse.tile as tile
from concourse.bass_utils import run_bass_kernel_spmd

B, N, F = 64, 1024, 64
NCORES = 8
BPC = B // NCORES  # batches per core
NT = N // 128      # 128-row chunks per batch
AF = mybir.AluOpType
dt = mybir.dt

BW = NT * F                    # 512 elems per batch per partition (tiled view)
EMB_W = NT * F                 # 512
MASK_W = BPC * NT              # 64
ADJ_W = NT * BPC + BPC - 1     # 71 (padded for the shifted lhsT slices)
CONST_W = EMB_W + MASK_W + ADJ_W

_CACHE = {}


def _emb_table() -> np.ndarray:
    """PositionalEncoding1D table, truncated to F channels, float32-faithful."""
    channels = ((N + 1) // 2) * 2
    exps = np.arange(0, channels, 2, dtype=np.float32) / np.float32(channels)
    inv_freq = np.float32(1.0) / np.power(np.float32(10000.0), exps, dtype=np.float32)
    pos = np.arange(N, dtype=np.float32)
    sin_inp = pos[:, None] * inv_freq[None, :]          # [N, channels/2]
    emb = np.concatenate(
        [np.sin(sin_inp), np.cos(sin_inp)], axis=-1
    ).astype(np.float32)[:, :F]                          # [N, F]
    return np.ascontiguousarray(emb)


def _build_program():
    nc = bacc.Bacc(
        "TRN2",
        target_bir_lowering=False,
        debug=False,
        enable_asserts=False,
        num_devices=NCORES,
    )
    nodes_in = nc.dram_tensor("nodes_s", (BPC * N, F), dt.float32, kind="ExternalInput").ap()
    # two contiguous const tensors: the small mask+adj strip loads first and
    # unblocks the matmuls / mask-mul well before the emb table lands
    consts_in = nc.dram_tensor("constt_s", (128, MASK_W + ADJ_W), dt.float32, kind="ExternalInput").ap()
    conste_in = nc.dram_tensor("constt_e", (128, EMB_W), dt.float32, kind="ExternalInput").ap()

    nodes_out = nc.dram_tensor("nodes_out", (BPC * N, F), dt.float32, kind="ExternalOutput").ap()
    agg_out = nc.dram_tensor("agg_out", (4, (BPC // 4) * F), dt.float32, kind="ExternalOutput").ap()

    # tiled [128, BPC*NT*F] views of the node tensors (2 KiB runs per partition)
    nodes_in_t = nodes_in.rearrange("(p r) f -> p (r f)", p=128)
    nodes_out_t = nodes_out.rearrange("(p r) f -> p (r f)", p=128)

    with tile.TileContext(nc) as tc, ExitStack() as ctx:
        consts = ctx.enter_context(tc.tile_pool(name="consts", bufs=1))
        big = ctx.enter_context(tc.tile_pool(name="big", bufs=1))
        psum = ctx.enter_context(tc.tile_pool(name="psum", bufs=1, space="PSUM"))

        csb_s = consts.tile([128, MASK_W + ADJ_W], dt.float32, tag="csb_s")
        nc.scalar.dma_start(csb_s[:], consts_in[:])
        csb_e = consts.tile([128, EMB_W], dt.float32, tag="csb_e")
        nc.scalar.dma_start(csb_e[:], conste_in[:])
        embsb = csb_e[:, :]
        masksb = csb_s[:, 0:MASK_W]
        adjsb = csb_s[:, MASK_W:MASK_W + ADJ_W]

        # all 8 batches' nodes in one [128, 4096] tile; per-batch loads
        # spread over the sync and scalar HWDGE queues (sync takes one more
        # since scalar also carries the consts) so batches arrive early and
        # compute pipelines behind the load stream
        nt_all = big.tile([128, BPC * BW], dt.float32, tag="nt_all")
        load_eng = {0: nc.sync, 1: nc.scalar, 2: nc.sync, 3: nc.scalar,
                    4: nc.sync, 5: nc.scalar, 6: nc.sync, 7: nc.sync}
        for b in range(BPC):
            load_eng[b].dma_start(
                nt_all[:, b * BW:(b + 1) * BW], nodes_in_t[:, b * BW:(b + 1) * BW]
            )

        # agg regions in 4 TensorE column tiles (128x32 mode): batch b uses
        # array tile b%4 writing psum partitions 32*(b%4)+(0..7), columns
        # (b//4)*64 of that quadrant's own psum bank; accumulate over chunks.
        psum_qs = [
            psum.tile([128, (BPC // 4) * F], dt.float32, tag=f"aggq{q}", name=f"aggq{q}")
            for q in range(4)
        ]
        for b in range(BPC):
            q = 32 * (b % 4)
            g = (b // 4) * F
            for t in range(NT):
                c = t * BPC + b
                nc.tensor.matmul(
                    psum_qs[b % 4][q:q + BPC, g:g + F],
                    lhsT=adjsb[:, c:c + BPC],
                    rhs=nt_all[:, b * BW + t * F: b * BW + (t + 1) * F],
                    start=(t == 0),
                    stop=(t == NT - 1),
                    tile_position=(0, q),
                )

        # masked posenc + add + store, pipelined per batch pair on DVE:
        #   tmp[p, b, t, f] = emb[p, t, f] * mask[p, b, t]; tmp += nodes; store
        tmp_all = big.tile([128, BPC * BW], dt.float32, tag="tmp_all")
        emb_b = bass.AP(
            csb_e.tensor, embsb.offset,
            [[EMB_W, 128], [0, 2], [F, NT], [1, F]],
        )
        for h in range(4):
            mask_b = (
                masksb[:, h * 2 * NT:(h + 1) * 2 * NT]
                .rearrange("p (b t) -> p b t", b=2)
                .to_broadcast([128, 2, NT, F])
            )
            out_b = (
                tmp_all[:, h * 2 * BW:(h + 1) * 2 * BW]
                .rearrange("p (b t f) -> p b t f", b=2, t=NT)
            )
            nc.vector.tensor_tensor(out_b, emb_b, mask_b, AF.mult)
            for b in (2 * h, 2 * h + 1):
                nc.vector.tensor_add(
                    tmp_all[:, b * BW:(b + 1) * BW],
                    tmp_all[:, b * BW:(b + 1) * BW],
                    nt_all[:, b * BW:(b + 1) * BW],
                )
                eng = nc.sync if b % 2 == 0 else nc.scalar
                eng.dma_start(
                    nodes_out_t[:, b * BW:(b + 1) * BW],
                    tmp_all[:, b * BW:(b + 1) * BW],
                )

        # agg strips live at psum partitions {0,32,64,96} of the 4 quadrant
        # banks: one ACT row-copy per quadrant (fires once that quadrant's
        # two groups finish) + one gather DMA
        aggsb = consts.tile([128, (BPC // 4) * F], dt.float32, tag="aggsb")
        for qi in range(4):
            q = 32 * qi
            nc.scalar.copy(aggsb[q:q + 1, :], psum_qs[qi][q:q + 1, :])
        agg_src = aggsb[:].rearrange("(q r) c -> q r c", r=32)[:, 0:1, :]
        nc.sync.dma_start(agg_out[:, :], agg_src)

    nc.compile()
    return nc


def get_program():
    if "nc" not in _CACHE:
        _CACHE["nc"] = _build_program()
    return _CACHE["nc"]


def _host_prep(x, nodes, adj, weights, num_nodes):
    """Wrap-overflow handling + per-core input marshalling."""
    nn0 = np.asarray(num_nodes)
    nn = nn0.astype(np.int64)
    of = (nn + 1) > N
    adj_eff, wts_eff, nodes_eff = adj, weights, nodes
    if of.any():  # structurally dead for randint(0, N) inputs; kept for fidelity
        nodes_w = nodes.copy()
        nodes_w[:, 0] = 0.0
        nodes_w = np.roll(nodes_w, -1, axis=1)
        adj_w = adj.copy()
        adj_w[:, 0, :] = 0.0
        adj_w[:, :, 0] = 0.0
        adj_w = np.roll(adj_w, (-1, -1), axis=(1, 2))
        wts_w = weights.copy()
        wts_w[:, 0, :] = 0.0
        wts_w[:, :, 0] = 0.0
        wts_w = np.roll(wts_w, (-1, -1), axis=(1, 2))
        m3 = of[:, None, None]
        nodes_eff = np.ascontiguousarray(np.where(m3, nodes_w, nodes))
        adj_eff = np.ascontiguousarray(np.where(m3, adj_w, adj))
        wts_eff = np.ascontiguousarray(np.where(m3, wts_w, weights))
        nn = np.where(of, nn - 1, nn)

    emb = _emb_table()
    b_idx = np.arange(B)
    adjrow = np.ascontiguousarray(adj_eff[b_idx, nn])            # [B, N]
    adj_nn = adjrow[b_idx, nn].astype(np.float32)                # adj[b, nn, nn]
    node_nn = nodes_eff[b_idx, nn].astype(np.float32)            # [B, F]
    corr = adj_nn[:, None] * (x.astype(np.float32) - node_nn)    # [B, F]
    xe = (x.astype(np.float32) + emb[nn]).astype(np.float32)     # [B, F]
    maskf = (np.arange(N)[None, :] <= nn[:, None]).astype(np.float32)  # [B, N]

    embt = emb.reshape(NT, 128, F).transpose(1, 0, 2).reshape(128, NT * F)

    in_maps = []
    for c in range(NCORES):
        s = slice(c * BPC, (c + 1) * BPC)
        # tiled: row p*64 + b*8 + t  <->  nodes[b, t*128 + p, :]
        nodes_tiled = np.ascontiguousarray(
            nodes_eff[s].astype(np.float32, copy=False)
            .reshape(BPC, NT, 128, F).transpose(2, 0, 1, 3).reshape(BPC * N, F)
        )
        maskt = (
            maskf[s].reshape(BPC, NT, 128).transpose(2, 0, 1).reshape(128, BPC * NT)
        )
        adjt = np.concatenate([
            adjrow[s].reshape(BPC, NT, 128).transpose(2, 1, 0)
            .reshape(128, NT * BPC).astype(np.float32, copy=False),
            np.zeros((128, BPC - 1), np.float32),
        ], axis=1)
        in_maps.append({
            "nodes_s": nodes_tiled,
            "constt_s": np.ascontiguousarray(
                np.concatenate([maskt, adjt], axis=1, dtype=np.float32)
            ),
            "constt_e": np.ascontiguousarray(embt.astype(np.float32)),
        })
    return in_maps, adj_eff, wts_eff, nn, nn0, corr, xe


def kernel(x, nodes, adj, weights, W, num_nodes, _run_kwargs=None):
    x = np.asarray(x)
    nodes = np.asarray(nodes)
    adj = np.asarray(adj)
    weights = np.asarray(weights)
    W = np.asarray(W).astype(np.float32, copy=False)
    in_maps, adj_eff, wts_eff, nn, nn0, corr, xe = _host_prep(
        x, nodes, adj, weights, num_nodes
    )

    nc = get_program()
    res = run_bass_kernel_spmd(
        nc, in_maps, core_ids=list(range(NCORES)), **(_run_kwargs or {})
    )

    nodes_out = np.empty((B, N, F), dtype=np.float32)
    agg = np.empty((B, F), dtype=np.float32)
    for c in range(NCORES):
        s = slice(c * BPC, (c + 1) * BPC)
        nodes_out[s] = (
            res.results[c]["nodes_out"]
            .reshape(128, BPC, NT, F).transpose(1, 2, 0, 3).reshape(BPC, N, F)
        )
        ao = res.results[c]["agg_out"]  # [4, 2F], batch b at [b%4, (b//4)*F:]
        for b in range(BPC):
            g = (b // 4) * F
            agg[c * BPC + b] = ao[b % 4, g:g + F]

    # scatter: row num_nodes[b] <- x[b] + posenc[nn_b]
    nodes_out[np.arange(B), nn] = xe
    mx = np.tanh((agg + corr) @ W).astype(np.float32)
    nn_out = (nn + 1).astype(nn0.dtype)

    out = (mx, nodes_out, adj_eff, wts_eff, nn_out)
    if _run_kwargs:
        return out, res
    return out


# revision 27
# speedup vs baseline: 1.0892x; 1.0892x over previous
"""Trainium2 Bass kernel for nn_DenseGCM (scatter_memory).

Reference semantics (B=64, N=1024, F=64):
    of = (num_nodes + 1) > N            # wrap_overflow -- structurally dead:
                                        # num_nodes ~ randint(0, N) <= N-1
    nodes_in  = nodes with row num_nodes[b] <- x[b]
    nodes_out = nodes_in + posenc * (n <= num_nodes[b])
    agg       = adj @ nodes_in
    mx        = tanh(agg @ W)[b, num_nodes[b]]
    returns (mx, nodes_out, adj, weights, num_nodes + 1)

Only one row of the [B,N,N]x[B,N,F] einsum is observable through mx:
    mx[b] = tanh((adj[b, nn_b, :] @ nodes_in[b]) @ W)
so the device computes that row's aggregation plus the full masked
positional-encoding add over nodes.  adj / weights are exact passthroughs;
the tiny (agg+corr)@W -> tanh tail and the 64-row scatter are applied on
the host during unsharding.

Sharding: pure data parallel, 8 batches per core across 8 NeuronCores.
All data-dependent quantities (mask columns, adj rows) are passed as
per-core input tensors so one SPMD program serves all cores.

Device-side layout: nodes are host-pre-tiled so each DMA moves 2-4 KiB
contiguous runs per partition (line rate) while keeping partition = node
row within a 128-chunk (what the matmul contraction needs).  DRAM row
r = p*64 + b*8 + t of the [8192, 64] tensor holds nodes[b, t*128+p, :].

The per-batch aggregation uses the adjacency rows as the *stationary*
matmul operand ([128, 8]: ldweights cost scales with columns) and node
chunks as moving operands, spread over 4 concurrent TensorE column tiles
(128x32 array mode, tile_position).  The lhsT slice for (b, t) starts at
adjt column t*8+b, putting batch b's adjrow in lhsT column 0, so every
psum region's valid row is its row 0 (at partitions {0,32,64,96}).
"""

from contextlib import ExitStack

import numpy as np

import concourse.bacc as bacc
import concourse.bass as bass
import concourse.mybir as mybir
import concourse.tile as tile
from concourse.bass_utils import run_bass_kernel_spmd

B, N, F = 64, 1024, 64
NCORES = 8
BPC = B // NCORES  # batches per core
NT = N // 128      # 128-row chunks per batch
AF = mybir.AluOpType
dt = mybir.dt

BW = NT * F                    # 512 elems per batch per partition (tiled view)
EMB_W = NT * F                 # 512
MASK_W = BPC * NT              # 64
ADJ_W = NT * BPC + BPC - 1     # 71 (padded for the shifted lhsT slices)
CONST_W = EMB_W + MASK_W + ADJ_W

_CACHE = {}


def _emb_table() -> np.ndarray:
    """PositionalEncoding1D table, truncated to F channels, float32-faithful."""
    channels = ((N + 1) // 2) * 2
    exps = np.arange(0, channels, 2, dtype=np.float32) / np.float32(channels)
    inv_freq = np.float32(1.0) / np.power(np.float32(10000.0), exps, dtype=np.float32)
    pos = np.arange(N, dtype=np.float32)
    sin_inp = pos[:, None] * inv_freq[None, :]          # [N, channels/2]
    emb = np.concatenate(
        [np.sin(sin_inp), np.cos(sin_inp)], axis=-1
    ).astype(np.float32)[:, :F]                          # [N, F]
    return np.ascontiguousarray(emb)


def _build_program():
    nc = bacc.Bacc(
        "TRN2",
        target_bir_lowering=False,
        debug=False,
        enable_asserts=False,
        num_devices=NCORES,
    )
    nodes_in = nc.dram_tensor("nodes_s", (BPC * N, F), dt.float32, kind="ExternalInput").ap()
    const_in = nc.dram_tensor("constt", (128, CONST_W), dt.float32, kind="ExternalInput").ap()

    nodes_out = nc.dram_tensor("nodes_out", (BPC * N, F), dt.float32, kind="ExternalOutput").ap()
    agg_out = nc.dram_tensor("agg_out", (4, (BPC // 4) * F), dt.float32, kind="ExternalOutput").ap()

    # tiled [128, BPC*NT*F] views of the node tensors (2 KiB runs per partition)
    nodes_in_t = nodes_in.rearrange("(p r) f -> p (r f)", p=128)
    nodes_out_t = nodes_out.rearrange("(p r) f -> p (r f)", p=128)

    with tile.TileContext(nc) as tc, ExitStack() as ctx:
        consts = ctx.enter_context(tc.tile_pool(name="consts", bufs=1))
        big = ctx.enter_context(tc.tile_pool(name="big", bufs=1))
        psum = ctx.enter_context(tc.tile_pool(name="psum", bufs=1, space="PSUM"))

        # one DMA for emb|mask|adj, on the scalar (ACT) HWDGE queue
        csb = consts.tile([128, CONST_W], dt.float32, tag="csb")
        nc.scalar.dma_start(csb[:], const_in[:])
        embsb = csb[:, 0:EMB_W]
        masksb = csb[:, EMB_W:EMB_W + MASK_W]
        adjsb = csb[:, EMB_W + MASK_W:CONST_W]

        # all 8 batches' nodes in one [128, 4096] tile; per-batch loads
        # spread over the sync and scalar HWDGE queues (sync takes one more
        # since scalar also carries the consts) so batches arrive early and
        # compute pipelines behind the load stream
        nt_all = big.tile([128, BPC * BW], dt.float32, tag="nt_all")
        load_eng = {0: nc.sync, 1: nc.scalar, 2: nc.sync, 3: nc.scalar,
                    4: nc.sync, 5: nc.scalar, 6: nc.sync, 7: nc.scalar}
        for b in range(BPC):
            load_eng[b].dma_start(
                nt_all[:, b * BW:(b + 1) * BW], nodes_in_t[:, b * BW:(b + 1) * BW]
            )

        # agg regions in 4 TensorE column tiles (128x32 mode): batch b uses
        # array tile b%4 writing psum partitions 32*(b%4)+(0..7), columns
        # (b//4)*64 of that quadrant's own psum bank; accumulate over chunks.
        psum_qs = [
            psum.tile([128, (BPC // 4) * F], dt.float32, tag=f"aggq{q}", name=f"aggq{q}")
            for q in range(4)
        ]
        for b in range(BPC):
            q = 32 * (b % 4)
            g = (b // 4) * F
            for t in range(NT):
                c = t * BPC + b
                nc.tensor.matmul(
                    psum_qs[b % 4][q:q + BPC, g:g + F],
                    lhsT=adjsb[:, c:c + BPC],
                    rhs=nt_all[:, b * BW + t * F: b * BW + (t + 1) * F],
                    start=(t == 0),
                    stop=(t == NT - 1),
                    tile_position=(0, q),
                )

        # masked posenc + add + store, pipelined per batch pair on DVE:
        #   tmp[p, b, t, f] = emb[p, t, f] * mask[p, b, t]; tmp += nodes; store
        tmp_all = big.tile([128, BPC * BW], dt.float32, tag="tmp_all")
        emb_b = bass.AP(
            csb.tensor, embsb.offset,
            [[CONST_W, 128], [0, 2], [F, NT], [1, F]],
        )
        for h in range(4):
            mask_b = (
                masksb[:, h * 2 * NT:(h + 1) * 2 * NT]
                .rearrange("p (b t) -> p b t", b=2)
                .to_broadcast([128, 2, NT, F])
            )
            out_b = (
                tmp_all[:, h * 2 * BW:(h + 1) * 2 * BW]
                .rearrange("p (b t f) -> p b t f", b=2, t=NT)
            )
            nc.vector.tensor_tensor(out_b, emb_b, mask_b, AF.mult)
            for b in (2 * h, 2 * h + 1):
                nc.vector.tensor_add(
                    tmp_all[:, b * BW:(b + 1) * BW],
                    tmp_all[:, b * BW:(b + 1) * BW],
                    nt_all[:, b * BW:(b + 1) * BW],
                )
                eng = nc.sync if b % 2 == 0 else nc.scalar
                eng.dma_start(
                    nodes_out_t[:, b * BW:(b + 1) * BW],
                    tmp_all[:, b * BW:(b + 1) * BW],
                )

        # agg strips live at psum partitions {0,32,64,96} of the 4 quadrant
        # banks: one ACT row-copy per quadrant (fires once that quadrant's
        # two groups finish) + one gather DMA
        aggsb = consts.tile([128, (BPC // 4) * F], dt.float32, tag="aggsb")
        for qi in range(4):
            q = 32 * qi
            nc.scalar.copy(aggsb[q:q + 1, :], psum_qs[qi][q:q + 1, :])
        agg_src = aggsb[:].rearrange("(q r) c -> q r c", r=32)[:, 0:1, :]
        nc.sync.dma_start(agg_out[:, :], agg_src)

    nc.compile()
    return nc


def get_program():
    if "nc" not in _CACHE:
        _CACHE["nc"] = _build_program()
    return _CACHE["nc"]


def _host_prep(x, nodes, adj, weights, num_nodes):
    """Wrap-overflow handling + per-core input marshalling."""
    nn0 = np.asarray(num_nodes)
    nn = nn0.astype(np.int64)
    of = (nn + 1) > N
    adj_eff, wts_eff, nodes_eff = adj, weights, nodes
    if of.any():  # structurally dead for randint(0, N) inputs; kept for fidelity
        nodes_w = nodes.copy()
        nodes_w[:, 0] = 0.0
        nodes_w = np.roll(nodes_w, -1, axis=1)
        adj_w = adj.copy()
        adj_w[:, 0, :] = 0.0
        adj_w[:, :, 0] = 0.0
        adj_w = np.roll(adj_w, (-1, -1), axis=(1, 2))
        wts_w = weights.copy()
        wts_w[:, 0, :] = 0.0
        wts_w[:, :, 0] = 0.0
        wts_w = np.roll(wts_w, (-1, -1), axis=(1, 2))
        m3 = of[:, None, None]
        nodes_eff = np.ascontiguousarray(np.where(m3, nodes_w, nodes))
        adj_eff = np.ascontiguousarray(np.where(m3, adj_w, adj))
        wts_eff = np.ascontiguousarray(np.where(m3, wts_w, weights))
        nn = np.where(of, nn - 1, nn)

    emb = _emb_table()
    b_idx = np.arange(B)
    adjrow = np.ascontiguousarray(adj_eff[b_idx, nn])            # [B, N]
    adj_nn = adjrow[b_idx, nn].astype(np.float32)                # adj[b, nn, nn]
    node_nn = nodes_eff[b_idx, nn].astype(np.float32)            # [B, F]
    corr = adj_nn[:, None] * (x.astype(np.float32) - node_nn)    # [B, F]
    xe = (x.astype(np.float32) + emb[nn]).astype(np.float32)     # [B, F]
    maskf = (np.arange(N)[None, :] <= nn[:, None]).astype(np.float32)  # [B, N]

    embt = emb.reshape(NT, 128, F).transpose(1, 0, 2).reshape(128, NT * F)

    in_maps = []
    for c in range(NCORES):
        s = slice(c * BPC, (c + 1) * BPC)
        # tiled: row p*64 + b*8 + t  <->  nodes[b, t*128 + p, :]
        nodes_tiled = np.ascontiguousarray(
            nodes_eff[s].astype(np.float32, copy=False)
            .reshape(BPC, NT, 128, F).transpose(2, 0, 1, 3).reshape(BPC * N, F)
        )
        maskt = (
            maskf[s].reshape(BPC, NT, 128).transpose(2, 0, 1).reshape(128, BPC * NT)
        )
        adjt = np.concatenate([
            adjrow[s].reshape(BPC, NT, 128).transpose(2, 1, 0)
            .reshape(128, NT * BPC).astype(np.float32, copy=False),
            np.zeros((128, BPC - 1), np.float32),
        ], axis=1)
        in_maps.append({
            "nodes_s": nodes_tiled,
            "constt": np.ascontiguousarray(
                np.concatenate([embt, maskt, adjt], axis=1, dtype=np.float32)
            ),
        })
    return in_maps, adj_eff, wts_eff, nn, nn0, corr, xe


def kernel(x, nodes, adj, weights, W, num_nodes, _run_kwargs=None):
    x = np.asarray(x)
    nodes = np.asarray(nodes)
    adj = np.asarray(adj)
    weights = np.asarray(weights)
    W = np.asarray(W).astype(np.float32, copy=False)
    in_maps, adj_eff, wts_eff, nn, nn0, corr, xe = _host_prep(
        x, nodes, adj, weights, num_nodes
    )

    nc = get_program()
    res = run_bass_kernel_spmd(
        nc, in_maps, core_ids=list(range(NCORES)), **(_run_kwargs or {})
    )

    nodes_out = np.empty((B, N, F), dtype=np.float32)
    agg = np.empty((B, F), dtype=np.float32)
    for c in range(NCORES):
        s = slice(c * BPC, (c + 1) * BPC)
        nodes_out[s] = (
            res.results[c]["nodes_out"]
            .reshape(128, BPC, NT, F).transpose(1, 2, 0, 3).reshape(BPC, N, F)
        )
        ao = res.results[c]["agg_out"]  # [4, 2F], batch b at [b%4, (b//4)*F:]
        for b in range(BPC):
            g = (b // 4) * F
            agg[c * BPC + b] = ao[b % 4, g:g + F]

    # scatter: row num_nodes[b] <- x[b] + posenc[nn_b]
    nodes_out[np.arange(B), nn] = xe
    mx = np.tanh((agg + corr) @ W).astype(np.float32)
    nn_out = (nn + 1).astype(nn0.dtype)

    out = (mx, nodes_out, adj_eff, wts_eff, nn_out)
    if _run_kwargs:
        return out, res
    return out
